# revision 1
# baseline (speedup 1.0000x reference)
"""Trainium2 Bass kernel for nn_BasicTT (TT-decomposed 3-layer MLP + log_softmax).

Strategy (8-way batch data parallelism, b=256 per core):
  Host prep (numpy):
    - Merge layer-1 TT cores 3,4,5 -> lhsT_A [K=512=(n3,n4,n5), 128=(r2,m3,m4,m5)]
      (right-to-left TT contraction: contract the big input dims first at
      full tensor-engine utilization)
    - Merge layer-1 cores 1,2 (+ layer-1 bias, smuggled in pad rows) ->
      lhsT_B [128=(g=(r2,m3h), j32=(n1,n2)pad), 64=(m3h,m1,m2)]
    - Layer 2 and 3 TT weights densified: W2 [64,2048], W3 [32,64]
    - Final linear reduced to the logit difference d = (W[1]-W[0])@h3 + bld;
      log_softmax = [-softplus(d), -softplus(-d)]
    - x pre-transposed per core to xT [512, b*24] (c-major) so phase-A
      loads are contiguous
  Device (per core, all fp32 / float32r matmuls):
    - Phase A: 4 accumulating K=128 matmuls -> psum [128, (b16, 24|8 pad)]
    - delta constants written into psum pad cols; DVE 32x32 stream-transpose
      moves (n1,n2) to partitions and turns pad cols into bias-activating
      rows for phase B
    - Phase B: single matmul (bias included via pad rows) -> h1 pre-relu
    - relu-split -> h1 [128=(m3l,m3h,m1,m2), (b,16=(m4,m5))]
    - L2: 16 accumulating matmuls -> relu(+b2) -> h2 [64,b]
    - L3: matmul -> relu(+b3) -> h3 [32,b]; d-matmul -> softplus tail
"""
import os
import numpy as np

NCORES = 8
B = 2048
BLOC = B // NCORES  # 256

_prog_cache = {}


# ---------------------------------------------------------------------------
# Host-side weight preparation
# ---------------------------------------------------------------------------
def _tt_full_matrix(cores):
    """Dense matrix W [prod(m), prod(n)] of a TT layer, matching the
    reference tt_linear index convention (input flat over n row-major,
    output flat over m row-major)."""
    n = 1
    for G in cores:
        n *= G.shape[2]
    x = np.eye(n)
    b = n
    z = x.reshape(b, 1, -1)
    for G in cores:
        r0, m, nn_, r1 = G.shape
        z4 = z.reshape(b, r0, nn_, -1)
        z = np.einsum('brns,rmnq->bqsm', z4, G).reshape(b, r1, -1)
    return z.reshape(b, -1).T


def _build_host_tensors(p):
    f64 = {k: np.asarray(v, np.float64) for k, v in p.items()}

    # G345 = l1c2 (r2,m3,n3,r3) * l1c3 (r3,m4,n4,r4) * l1c4 (r4,m5,n5,1)
    g34 = np.einsum('amcb,bndq->amncdq', f64['l1c2'], f64['l1c3'])
    g345 = np.einsum('amncdq,qpe->amnpcde', g34, f64['l1c4'][:, :, :, 0])
    # g345[r2,m3,m4,m5,n3,n4,n5] -> lhsT_A [(n3,n4,n5)=512, (r2,m3,m4,m5)=128]
    lhsT_A = g345.transpose(4, 5, 6, 0, 1, 2, 3).reshape(512, 128)
    # pack K-tiles side by side: gA[c128, k*128 + p] = lhsT_A[k*128+c, p]
    gA = np.ascontiguousarray(
        lhsT_A.reshape(4, 128, 128).transpose(1, 0, 2).reshape(128, 512))

    # G12 = l1c0 (1,m1,n1,r1) * l1c1 (r1,m2,n2,r2) -> g12[n1,n2,r2,m1,m2]
    g12 = np.einsum('mar,rnbq->abqmn', f64['l1c0'][0], f64['l1c1'])
    b1 = f64['b1']  # (m1,m2,m3,m4,m5) = (8,4,4,4,4)
    lhsT_B = np.zeros((128, 64))
    for r2 in range(2):
        for m3h in range(2):
            g = r2 * 2 + m3h
            for n1 in range(3):
                for n2 in range(8):
                    j = n1 * 8 + n2
                    for m1 in range(8):
                        for m2 in range(4):
                            lhsT_B[g * 32 + j, m3h * 32 + m1 * 4 + m2] = \
                                g12[n1, n2, r2, m1, m2]
    # bias rows: row (g, 24+j') fires for u = 8g+j' (delta pattern sits in
    # the sg staging-buffer pad cols, written once at startup)
    for g in range(4):
        for jp in range(8):
            u = 8 * g + jp
            m3l, m4, m5 = u >> 4, (u >> 2) & 3, u & 3
            for m3h in range(2):
                for m1 in range(8):
                    for m2 in range(4):
                        m3 = m3h * 2 + m3l
                        lhsT_B[g * 32 + 24 + jp, m3h * 32 + m1 * 4 + m2] = \
                            b1[m1, m2, m3, m4, m5]
    # delta pattern for the sg pads [128, (b16, 8)]:
    # row p=(g,u) has 1.0 at pad col j' iff u == 8g+j'
    dltrep = np.zeros((128, 8))
    for gg in range(4):
        for u in range(32):
            jp = u - 8 * gg
            if 0 <= jp < 8:
                dltrep[gg * 32 + u, jp] = 1.0
    dltrep = np.tile(dltrep, (1, 16))  # [128, 128]

    # dense layer 2/3
    W2 = _tt_full_matrix([f64['l2c0'], f64['l2c1'], f64['l2c2'],
                          f64['l2c3'], f64['l2c4']])  # [64, 2048]
    W3 = _tt_full_matrix([f64['l3c0'], f64['l3c1'], f64['l3c2'],
                          f64['l3c3'], f64['l3c4']])  # [32, 64]
    # g2 [128=(m3l,m3h,m1,m2), (v16, f64)]
    g2 = np.zeros((128, 16, 64))
    for pp in range(128):
        m3l, m3h = pp >> 6, (pp >> 5) & 1
        m1, m2 = (pp >> 2) & 7, pp & 3
        m3 = m3h * 2 + m3l
        for v in range(16):
            m4, m5 = v >> 2, v & 3
            flat = (((m1 * 4 + m2) * 4 + m3) * 4 + m4) * 4 + m5
            g2[pp, v, :] = W2[:, flat]
    g2 = g2.reshape(128, 1024)

    f32 = np.float32
    # fp16 matmul consts, one DMA:
    #   gA 0:512 | g2 512:1536 | gB 1536:1600 | g3 1600:1632 (rows 0:64)
    #   | wd 1632:1633 (rows 0:32)
    cstH = np.zeros((128, 1633), np.float16)
    cstH[:, 0:512] = gA.astype(np.float16)
    cstH[:, 512:1536] = g2.astype(np.float16)
    cstH[:, 1536:1600] = lhsT_B.astype(np.float16)
    cstH[0:64, 1600:1632] = W3.T.astype(np.float16)
    cstH[0:32, 1632:1633] = (f64['W'][1] - f64['W'][0]).reshape(32, 1) \
        .astype(np.float16)
    # f32 consts: dltrep 0:128 | b2 128:129 (rows 0:64)
    #   | b3 129:130 (rows 0:32) | bld 130:131 (row 0)
    cstF = np.zeros((128, 131), f32)
    cstF[:, 0:128] = dltrep
    cstF[0:64, 128:129] = f64['b2'].reshape(64, 1)
    cstF[0:32, 129:130] = f64['b3'].reshape(32, 1)
    cstF[0, 130] = f64['bl'][1] - f64['bl'][0]
    return dict(cstH=cstH, cstF=cstF)


def _make_xT(x_core):
    b = x_core.shape[0]
    xr = np.asarray(x_core, np.float32).reshape(b, 24, 512)
    xt = np.ascontiguousarray(xr.transpose(2, 0, 1).reshape(512, b * 24))
    return xt.astype(np.float16)


# ---------------------------------------------------------------------------
# Device program
# ---------------------------------------------------------------------------
def _build_program():
    if 'nc' in _prog_cache:
        return _prog_cache['nc']
    from contextlib import ExitStack
    import concourse.bacc as bacc
    import concourse.mybir as mybir
    import concourse.tile as tile

    F32R = mybir.dt.float32r
    F16 = mybir.dt.float16
    F32 = mybir.dt.float32
    AF = mybir.ActivationFunctionType

    nc = bacc.Bacc(None, target_bir_lowering=False)

    xT = nc.declare_dram_parameter("xT", [512, BLOC * 24], F16, isOutput=False)
    cstH = nc.declare_dram_parameter("cstH", [128, 1633], F16, isOutput=False)
    cstF = nc.declare_dram_parameter("cstF", [128, 131], F32, isOutput=False)
    y = nc.declare_dram_parameter("y", [BLOC, 2], F32, isOutput=True)

    with tile.TileContext(nc) as tc, ExitStack() as ctx:
        consts = ctx.enter_context(tc.tile_pool(name="consts", bufs=1))
        xpool = ctx.enter_context(tc.tile_pool(name="x", bufs=3))
        tpool = ctx.enter_context(tc.tile_pool(name="tb", bufs=3))
        h1pool = ctx.enter_context(tc.tile_pool(name="h1", bufs=1))
        spool = ctx.enter_context(tc.tile_pool(name="small", bufs=1))
        psA = ctx.enter_context(tc.tile_pool(name="psA", bufs=2, space="PSUM"))
        psB = ctx.enter_context(tc.tile_pool(name="psB", bufs=2, space="PSUM"))
        psT = ctx.enter_context(tc.tile_pool(name="psT", bufs=1, space="PSUM"))

        cH = consts.tile([128, 1633], F16, tag="cstH")
        nc.scalar.dma_start(cH[:, :], cstH[:, :])
        cF = consts.tile([128, 131], F32, tag="cstF")
        nc.scalar.dma_start(cF[:, :], cstF[:, :])
        gA_t = cH[:, 0:512]
        g2_t = cH[:, 512:1536]
        gB_t = cH[:, 1536:1600]
        g3_t = cH[0:64, 1600:1632]
        wd_t = cH[0:32, 1632:1633]
        dltrep_t = cF[:, 0:128]
        b2_t = cF[0:64, 128:129]
        b3_t = cF[0:32, 129:130]
        bld_t = cF[0:1, 130:131]

        h1 = h1pool.tile([128, BLOC * 16], F16)

        # persistent staging buffers, zeroed once: their pad cols (b, 24:32)
        # are never written afterwards and face zero rows of gB, but must
        # stay finite (NaN * 0 = NaN).
        sg_bufs = []
        for i in range(3):
            z = tpool.tile([128, 512], F16, tag=f"sg{i}")
            nc.vector.tensor_copy(
                z.rearrange("p (b j) -> p b j", j=32)[:, :, 24:32],
                dltrep_t.rearrange("p (b j) -> p b j", j=8))
            sg_bufs.append(z)

        for bc in range(BLOC // 32):  # b32 chunks
            b0 = bc * 32
            # xt layout [128, (k4, b32, 24)]: fully contiguous DMA
            xt = xpool.tile([128, 4 * 32 * 24], F16, tag="xt")
            nc.sync.dma_start(
                xt[:, :],
                xT.rearrange("(k p) c -> p k c", k=4)[:, :, b0 * 24:(b0 + 32) * 24])
            pss = [psA.tile([128, 384], F32, tag=f"psA{hf}",
                            name=f"psA{hf}_{bc}")
                   for hf in range(2)]
            for k in range(4):  # weights loaded once per k, two b16 matmuls
                for hf in range(2):
                    nc.tensor.matmul(
                        pss[hf][:, :],
                        gA_t[:, k * 128:(k + 1) * 128],
                        xt[:, k * 768 + hf * 384: k * 768 + (hf + 1) * 384],
                        start=(k == 0), stop=(k == 3))
            for hf in range(2):  # b16 halves
                ps = pss[hf]
                # stage psum->sbuf with (b,32) interleave (ScalarE);
                # pad cols 24:32 hold the delta pattern (bias rows of gB)
                sg = sg_bufs[(bc * 2 + hf) % 3]
                nc.scalar.activation(
                    sg.rearrange("p (b j) -> p b j", j=32)[:, :, 0:24],
                    ps.rearrange("p (b j) -> p b j", j=24),
                    AF.Copy)
                # 32x32 stream transpose: [(g,u),(b,j)] -> [(g,j),(b,u)]
                tb = tpool.tile([128, 512], F16, tag="tb")
                nc.vector.transpose(tb[:, :], sg[:, :])
                # phase B: layer-1 left side + bias (pad rows x delta)
                pb = psB.tile([64, 512], F32, tag="psB")
                nc.tensor.matmul(pb[:, :], gB_t[:, :], tb[:, :],
                                 start=True, stop=True)
                # relu + split (m3l) into h1 [128, (b,16)]
                pb3 = pb.rearrange("p (b u) -> p b u", u=32)
                dst = h1[:, (b0 + hf * 16) * 16:(b0 + hf * 16 + 16) * 16] \
                    .rearrange("p (b v) -> p b v", v=16)
                nc.vector.tensor_scalar_max(dst[0:64], pb3[:, :, 0:16], 0.0)
                nc.vector.tensor_scalar_max(dst[64:128], pb3[:, :, 16:32], 0.0)

        # ---- layer 2 ----
        p2 = psT.tile([64, BLOC], F32, tag="pt")
        h1v = h1.rearrange("p (b v) -> p v b", v=16)
        for v in range(16):
            nc.tensor.matmul(p2[:, :], g2_t[:, v * 64:(v + 1) * 64],
                             h1v[:, v, :], start=(v == 0), stop=(v == 15))
        h2 = spool.tile([64, BLOC], F16, tag="h2")
        nc.scalar.activation(h2[:, :], p2[:, :], AF.Relu, bias=b2_t[:, 0:1])
        # ---- layer 3 ----
        p3 = psT.tile([32, BLOC], F32, tag="pt")
        nc.tensor.matmul(p3[:, :], g3_t[:, :], h2[:, :], start=True, stop=True)
        h3 = spool.tile([32, BLOC], F16, tag="h3")
        nc.scalar.activation(h3[:, :], p3[:, :], AF.Relu, bias=b3_t[:, 0:1])
        # ---- logit diff + log_softmax ----
        pd = psT.tile([1, BLOC], F32, tag="pt")
        nc.tensor.matmul(pd[:, :], wd_t[:, :], h3[:, :], start=True, stop=True)
        # stable softplus: sp = relu(D) + ln(1 + exp(-|D|)), D = d + bld
        dpb = spool.tile([1, BLOC], F32, tag="dpb")
        nc.vector.tensor_scalar_add(dpb[:, :], pd[:, :], bld_t[0:1, 0:1])
        ng = spool.tile([1, BLOC], F32, tag="ng")
        nc.vector.tensor_scalar_mul(ng[:, :], dpb[:, :], -1.0)
        na = spool.tile([1, BLOC], F32, tag="na")
        nc.vector.tensor_tensor(na[:, :], dpb[:, :], ng[:, :],
                                op=mybir.AluOpType.min)
        ex = spool.tile([1, BLOC], F32, tag="ex")
        nc.scalar.activation(ex[:, :], na[:, :], AF.Exp)
        ln1 = spool.tile([1, BLOC], F32, tag="ln1")
        nc.scalar.activation(ln1[:, :], ex[:, :], AF.Ln, bias=1.0)
        rl = spool.tile([1, BLOC], F32, tag="rl")
        nc.scalar.activation(rl[:, :], dpb[:, :], AF.Relu)
        sp = spool.tile([1, BLOC], F32, tag="sp")
        nc.vector.tensor_add(sp[:, :], ln1[:, :], rl[:, :])
        out0 = spool.tile([1, BLOC], F32, tag="out0")
        nc.vector.tensor_scalar_mul(out0[:, :], sp[:, :], -1.0)
        out1 = spool.tile([1, BLOC], F32, tag="out1")
        nc.vector.tensor_sub(out1[:, :], dpb[:, :], sp[:, :])
        yT = y.rearrange("b i -> i b")
        nc.sync.dma_start(yT[0:1, :], out0[:, :])
        nc.sync.dma_start(yT[1:2, :], out1[:, :])

    nc.compile()
    _prog_cache['nc'] = nc
    return nc


# ---------------------------------------------------------------------------
# Entry point
# ---------------------------------------------------------------------------
def kernel(**inputs):
    from concourse.bass_utils import run_bass_kernel_spmd

    H = _build_host_tensors(inputs)
    x = np.asarray(inputs['x'], np.float32)
    nc = _build_program()

    in_maps = []
    for c in range(NCORES):
        m = dict(H)
        m['xT'] = _make_xT(x[c * BLOC:(c + 1) * BLOC])
        in_maps.append(m)

    trace = bool(os.environ.get('KERNEL_TRACE'))
    tmpdir = None
    if trace:
        tmpdir = os.environ.get('KERNEL_TRACE_DIR') or None
        if tmpdir:
            os.makedirs(tmpdir, exist_ok=True)
    res = run_bass_kernel_spmd(nc, in_maps, list(range(NCORES)),
                               trace=trace, tmpdir=tmpdir)
    kernel.last_results = res
    out = np.concatenate([res.results[c]['y'] for c in range(NCORES)], axis=0)
    return out.astype(np.float32)


if __name__ == '__main__':
    # smoke test with random inputs shaped per spec
    rng = np.random.default_rng(0)
    shapes = {
        'x': (B, 3, 8, 8, 8, 8),
        'l1c0': (1, 8, 3, 3), 'l1c1': (3, 4, 8, 2), 'l1c2': (2, 4, 8, 2),
        'l1c3': (2, 4, 8, 2), 'l1c4': (2, 4, 8, 1), 'b1': (8, 4, 4, 4, 4),
        'l2c0': (1, 4, 8, 2), 'l2c1': (2, 2, 4, 2), 'l2c2': (2, 2, 4, 2),
        'l2c3': (2, 2, 4, 2), 'l2c4': (2, 2, 4, 1), 'b2': (4, 2, 2, 2, 2),
        'l3c0': (1, 2, 4, 2), 'l3c1': (2, 2, 2, 2), 'l3c2': (2, 2, 2, 2),
        'l3c3': (2, 2, 2, 2), 'l3c4': (2, 2, 2, 1), 'b3': (2, 2, 2, 2, 2),
        'W': (2, 32), 'bl': (2,),
    }
    ins = {k: rng.standard_normal(v).astype(np.float32) * 0.3
           for k, v in shapes.items()}
    print(kernel(**ins)[:4])



# revision 9
# speedup vs baseline: 1.2462x; 1.2462x over previous
"""Trainium2 Bass kernel for nn_BasicTT (TT-decomposed 3-layer MLP + log_softmax).

Strategy (8-way batch data parallelism, b=256 per core):
  Host prep (numpy):
    - Merge layer-1 TT cores 3,4,5 -> lhsT_A [K=512=(n3,n4,n5), 128=(r2,m3,m4,m5)]
    - Merge layer-1 cores 1,2 (+ layer-1 bias in pad rows) -> lhsT_B [128, 64]
    - Layer 2 and 3 TT weights densified: g2 [128,(v16,64)], g3 [64,32]
    - Final linear reduced to the logit difference d = (W[1]-W[0])@h3 + bld
      (bld folded as a 33rd row of the d-matmul against a ones-row of h3);
      log_softmax = [-softplus(d), -softplus(-d)]
    - x pre-transposed per core to xT [512, b*24] fp16
  Device (per core), pipelined per b32 chunk:
    - warmup matmuls on dummy data bring the PE out of the cold p-state
      while the input DMA streams
    - Phase A: 8 accumulating K=128 fp16 matmuls -> 2 psum [128,(b16,24)]
    - ScalarE copies psum -> sg [128,(b32,32)] (pad cols hold the persistent
      bias-delta pattern); one DVE 32x32 stream-transpose [128,1024]
    - Phase B: 2 matmuls (bias via pad rows) -> psB [64,(b16,u32)]
    - relu-split into h1 [128,(b128,16)]: ScalarE takes the aligned half,
      DVE the partition-shifted half
    - per b128 half: L2 (16 acc matmuls) -> relu+b2 -> L3 -> relu+b3 ->
      d-matmul -> softplus tail -> contiguous y DMA
  Only Copy/Relu/Softplus activations are used (single ACT table load).
"""
import os
import numpy as np

NCORES = 8
B = 2048
BLOC = B // NCORES  # 256
NCHUNK = 8          # b32 chunks per core
BC = BLOC // NCHUNK  # 32

_prog_cache = {}


# ---------------------------------------------------------------------------
# Host-side weight preparation
# ---------------------------------------------------------------------------
def _tt_full_matrix(cores):
    """Dense matrix W [prod(m), prod(n)] of a TT layer, matching the
    reference tt_linear index convention."""
    n = 1
    for G in cores:
        n *= G.shape[2]
    x = np.eye(n)
    b = n
    z = x.reshape(b, 1, -1)
    for G in cores:
        r0, m, nn_, r1 = G.shape
        z4 = z.reshape(b, r0, nn_, -1)
        z = np.einsum('brns,rmnq->bqsm', z4, G).reshape(b, r1, -1)
    return z.reshape(b, -1).T


def _build_host_tensors(p):
    f64 = {k: np.asarray(v, np.float64) for k, v in p.items()}

    # G345 = l1c2 (r2,m3,n3,r3) * l1c3 (r3,m4,n4,r4) * l1c4 (r4,m5,n5,1)
    g34 = np.einsum('amcb,bndq->amncdq', f64['l1c2'], f64['l1c3'])
    g345 = np.einsum('amncdq,qpe->amnpcde', g34, f64['l1c4'][:, :, :, 0])
    # g345[r2,m3,m4,m5,n3,n4,n5] -> lhsT_A [(n3,n4,n5)=512, (r2,m3,m4,m5)=128]
    lhsT_A = g345.transpose(4, 5, 6, 0, 1, 2, 3).reshape(512, 128)
    gA = np.ascontiguousarray(
        lhsT_A.reshape(4, 128, 128).transpose(1, 0, 2).reshape(128, 512))

    # G12 = l1c0 (1,m1,n1,r1) * l1c1 (r1,m2,n2,r2) -> g12[n1,n2,r2,m1,m2]
    g12 = np.einsum('mar,rnbq->abqmn', f64['l1c0'][0], f64['l1c1'])
    b1 = f64['b1']  # (m1,m2,m3,m4,m5) = (8,4,4,4,4)
    lhsT_B = np.zeros((128, 64))
    for r2 in range(2):
        for m3h in range(2):
            g = r2 * 2 + m3h
            for n1 in range(3):
                for n2 in range(8):
                    j = n1 * 8 + n2
                    for m1 in range(8):
                        for m2 in range(4):
                            lhsT_B[g * 32 + j, m3h * 32 + m1 * 4 + m2] = \
                                g12[n1, n2, r2, m1, m2]
    # bias rows: row (g, 24+j') fires for u = 8g+j'
    for g in range(4):
        for jp in range(8):
            u = 8 * g + jp
            m3l, m4, m5 = u >> 4, (u >> 2) & 3, u & 3
            for m3h in range(2):
                for m1 in range(8):
                    for m2 in range(4):
                        m3 = m3h * 2 + m3l
                        lhsT_B[g * 32 + 24 + jp, m3h * 32 + m1 * 4 + m2] = \
                            b1[m1, m2, m3, m4, m5]
    # delta pattern for the sg pads, tiled over b32: row p=(g,u) has 1.0 at
    # pad col (b, j') iff u == 8g+j'
    dltrep = np.zeros((128, 8))
    for gg in range(4):
        for u in range(32):
            jp = u - 8 * gg
            if 0 <= jp < 8:
                dltrep[gg * 32 + u, jp] = 1.0
    dltrep = np.tile(dltrep, (1, BC))  # [128, 256]

    # dense layer 2/3
    W2 = _tt_full_matrix([f64['l2c0'], f64['l2c1'], f64['l2c2'],
                          f64['l2c3'], f64['l2c4']])  # [64, 2048]
    W3 = _tt_full_matrix([f64['l3c0'], f64['l3c1'], f64['l3c2'],
                          f64['l3c3'], f64['l3c4']])  # [32, 64]
    # g2 [128=(m3l,m3h,m1,m2), (v16, f64)]
    g2 = np.zeros((128, 16, 64))
    for pp in range(128):
        m3l, m3h = pp >> 6, (pp >> 5) & 1
        m1, m2 = (pp >> 2) & 7, pp & 3
        m3 = m3h * 2 + m3l
        for v in range(16):
            m4, m5 = v >> 2, v & 3
            flat = (((m1 * 4 + m2) * 4 + m3) * 4 + m4) * 4 + m5
            g2[pp, v, :] = W2[:, flat]
    g2 = g2.reshape(128, 1024)

    f32 = np.float32
    # fp16 matmul consts, one DMA:
    #   gA 0:512 | g2 512:1536 | gB 1536:1600 | g3 1600:1632 (rows 0:64)
    #   | wd 1632:1633 (rows 0:33; row 32 = bld for the h3 ones-row)
    cstH = np.zeros((128, 1633), np.float16)
    cstH[:, 0:512] = gA.astype(np.float16)
    cstH[:, 512:1536] = g2.astype(np.float16)
    cstH[:, 1536:1600] = lhsT_B.astype(np.float16)
    cstH[0:64, 1600:1632] = W3.T.astype(np.float16)
    cstH[0:32, 1632:1633] = (f64['W'][1] - f64['W'][0]).reshape(32, 1) \
        .astype(np.float16)
    cstH[32, 1632] = np.float16(f64['bl'][1] - f64['bl'][0])
    # f32 consts: dltrep 0:256 | b2 256:257 (rows 0:64) | b3 257:258 (rows 0:32)
    cstF = np.zeros((128, 258), f32)
    cstF[:, 0:256] = dltrep
    cstF[0:64, 256:257] = f64['b2'].reshape(64, 1)
    cstF[0:32, 257:258] = f64['b3'].reshape(32, 1)
    return dict(cstH=cstH, cstF=cstF)


def _make_xT(x_core):
    b = x_core.shape[0]
    xr = np.asarray(x_core, np.float32).reshape(b, 24, 512)
    xt = np.ascontiguousarray(xr.transpose(2, 0, 1).reshape(512, b * 24))
    return xt.astype(np.float16)


# ---------------------------------------------------------------------------
# Device program
# ---------------------------------------------------------------------------
def _build_program():
    if 'nc' in _prog_cache:
        return _prog_cache['nc']
    from contextlib import ExitStack
    import concourse.bacc as bacc
    import concourse.mybir as mybir
    import concourse.tile as tile

    F16 = mybir.dt.float16
    F32 = mybir.dt.float32
    AF = mybir.ActivationFunctionType

    nc = bacc.Bacc(None, target_bir_lowering=False)

    xT = nc.declare_dram_parameter("xT", [512, BLOC * 24], F16, isOutput=False)
    cstH = nc.declare_dram_parameter("cstH", [128, 1633], F16, isOutput=False)
    cstF = nc.declare_dram_parameter("cstF", [128, 258], F32, isOutput=False)
    y = nc.declare_dram_parameter("y", [BLOC, 2], F32, isOutput=True)

    with tile.TileContext(nc) as tc, ExitStack() as ctx:
        consts = ctx.enter_context(tc.tile_pool(name="consts", bufs=1))
        xpool = ctx.enter_context(tc.tile_pool(name="x", bufs=4))
        tpool = ctx.enter_context(tc.tile_pool(name="tb", bufs=3))
        h1pool = ctx.enter_context(tc.tile_pool(name="h1", bufs=1))
        spool = ctx.enter_context(tc.tile_pool(name="small", bufs=2))
        psA = ctx.enter_context(tc.tile_pool(name="psA", bufs=4, space="PSUM"))
        psB = ctx.enter_context(tc.tile_pool(name="psB", bufs=3, space="PSUM"))
        psT = ctx.enter_context(tc.tile_pool(name="psT", bufs=1, space="PSUM"))

        # consts: issue both DMAs from the Sync sequencer so they are not
        # gated behind ScalarE's ACT table load
        cH = consts.tile([128, 1633], F16, tag="cstH")
        nc.sync.dma_start(cH[:, :], cstH[:, :])
        cF = consts.tile([128, 258], F32, tag="cstF")
        nc.sync.dma_start(cF[:, :], cstF[:, :])
        gA_t = cH[:, 0:512]
        g2_t = cH[:, 512:1536]
        gB_t = cH[:, 1536:1600]
        g3_t = cH[0:64, 1600:1632]
        wd_t = cH[0:33, 1632:1633]
        dltrep_t = cF[:, 0:256]
        b2_t = cF[0:64, 256:257]
        b3_t = cF[0:32, 257:258]

        # PE warmup during the input-DMA window: dummy matmuls on a zeroed
        # tile lift the PE out of the cold p-state before real work arrives.
        wrm = consts.tile([128, 512], F16, tag="wrm")
        nc.vector.memset(wrm[:, :], 0.0)
        # dummy Sigmoid first: every activation used later (Abs, Sigmoid,
        # Relu, Copy) lives in the sigmoid table, so this pins the one and
        # only ACT table load to kernel start
        sgd = consts.tile([1, 1], F32, tag="sgd")
        nc.scalar.activation(sgd[:, :], wrm[0:1, 0:1], AF.Sigmoid)
        for w in range(8):
            pw = psA.tile([128, 384], F32, tag="psA", name=f"warm{w}")
            nc.tensor.matmul(pw[:, :], wrm[:, 0:128], wrm[:, 0:384],
                             start=True, stop=True)

        # persistent staging buffers: pad cols (b, 24:32) hold the bias
        # delta pattern, written once; data cols rewritten every chunk
        sg_bufs = []
        for i in range(3):
            z = tpool.tile([128, BC * 32], F16, tag=f"sg{i}", name=f"sg{i}")
            nc.gpsimd.tensor_copy(
                z.rearrange("p (b j) -> p b j", j=32)[:, :, 24:32],
                dltrep_t.rearrange("p (b j) -> p b j", j=8))
            sg_bufs.append(z)

        # h1 per b128 half [128=(m3l,m3h,m1,m2), (b128, v16)]
        h1h = [h1pool.tile([128, 128 * 16], F16, tag=f"h1{h}", name=f"h1{h}")
               for h in range(2)]
        # h3 per half [33, 128]: row 32 is the ones-row that folds bld in
        h3h = []
        for h in range(2):
            t = spool.tile([33, 128], F16, tag=f"h3{h}", name=f"h3{h}")
            nc.gpsimd.memset(t[32:33, :], 1.0)
            h3h.append(t)
        # y staging [1, (b256, i2)] f32, one contiguous DMA per half
        ystage = consts.tile([1, 512], F32, tag="ystage")

        for bc in range(NCHUNK):
            half = bc // (NCHUNK // 2)
            b0 = bc * BC
            # xt layout [128, (k4, b32, 24)]: fully contiguous DMA
            xt = xpool.tile([128, 4 * BC * 24], F16, tag="xt")
            nc.sync.dma_start(
                xt[:, :],
                xT.rearrange("(k p) c -> p k c", k=4)[:, :, b0 * 24:(b0 + BC) * 24])
            xt4 = xt.rearrange("p (k b j) -> p k b j", k=4, j=24)
            pss = [psA.tile([128, 384], F32, tag="psA", name=f"psA{bc}_{hf}")
                   for hf in range(2)]
            for k in range(4):  # weights loaded once per k, two b16 matmuls
                for hf in range(2):
                    nc.tensor.matmul(
                        pss[hf][:, :],
                        gA_t[:, k * 128:(k + 1) * 128],
                        xt4[:, k, hf * 16:(hf + 1) * 16, :],
                        start=(k == 0), stop=(k == 3))
            # stage psum->sbuf (ScalarE) with (b,32) interleave; pad cols
            # already hold the delta pattern (bias rows of gB)
            sg = sg_bufs[bc % 3]
            sg3 = sg.rearrange("p (b j) -> p b j", j=32)
            for hf in range(2):
                nc.scalar.activation(
                    sg3[:, hf * 16:(hf + 1) * 16, 0:24],
                    pss[hf].rearrange("p (b j) -> p b j", j=24),
                    AF.Copy)
            # 32x32 stream transpose: [(g,u),(b,j)] -> [(g,j),(b,u)]
            tb = tpool.tile([128, BC * 32], F16, tag="tb")
            nc.vector.transpose(tb[:, :], sg[:, :])
            dst = h1h[half][:, ((bc % 4) * 32) * 16:((bc % 4) * 32 + 32) * 16] \
                .rearrange("p (b v) -> p b v", v=16)
            for hf in range(2):
                # phase B: layer-1 left side + bias (pad rows x delta)
                pb = psB.tile([64, 512], F32, tag="psB", name=f"psB{bc}_{hf}")
                nc.tensor.matmul(pb[:, :], gB_t[:, :],
                                 tb[:, hf * 512:(hf + 1) * 512],
                                 start=True, stop=True)
                # relu + split (m3l) into h1: aligned half on ScalarE,
                # partition-shifted half on DVE
                pb3 = pb.rearrange("p (b u) -> p b u", u=32)
                d3 = dst[:, hf * 16:(hf + 1) * 16, :]
                nc.scalar.activation(d3[0:64], pb3[:, :, 0:16], AF.Relu)
                nc.vector.tensor_scalar_max(d3[64:128], pb3[:, :, 16:32], 0.0)

            if bc % (NCHUNK // 2) == (NCHUNK // 2) - 1:
                # ---- layers 2/3 + log_softmax tail for this b128 half ----
                p2 = psT.tile([64, 128], F32, tag="pt", name=f"p2_{half}")
                h1v = h1h[half].rearrange("p (b v) -> p v b", v=16)
                for v in range(16):
                    nc.tensor.matmul(p2[:, :], g2_t[:, v * 64:(v + 1) * 64],
                                     h1v[:, v, :], start=(v == 0),
                                     stop=(v == 15))
                h2 = spool.tile([64, 128], F16, tag=f"h2{half}", name=f"h2_{half}")
                nc.scalar.activation(h2[:, :], p2[:, :], AF.Relu,
                                     bias=b2_t[:, 0:1])
                p3 = psT.tile([32, 128], F32, tag="pt", name=f"p3_{half}")
                nc.tensor.matmul(p3[:, :], g3_t[:, :], h2[:, :],
                                 start=True, stop=True)
                nc.scalar.activation(h3h[half][0:32, :], p3[:, :], AF.Relu,
                                     bias=b3_t[:, 0:1])
                # logit diff (K=33 includes the bld ones-row)
                pd = psT.tile([1, 128], F32, tag="pt", name=f"pd_{half}")
                nc.tensor.matmul(pd[:, :], wd_t[:, :], h3h[half][:, :],
                                 start=True, stop=True)
                # log_softmax = [-softplus(d), d - softplus(d)] with
                # softplus(d) = relu(d) - ln(1-s), s = sigmoid(-|d|), and
                # -ln(1-s) ~= s + s^2/2 + s^3/3 + s^4/4 + s^5/5  (|err|<.5%)
                aT = spool.tile([1, 128], F32, tag=f"aT{half}", name=f"aT{half}")
                nc.scalar.activation(aT[:, :], pd[:, :], AF.Abs)
                sS = spool.tile([1, 128], F32, tag=f"sS{half}", name=f"sS{half}")
                nc.scalar.activation(sS[:, :], aT[:, :], AF.Sigmoid,
                                     scale=-1.0)
                rl = spool.tile([1, 128], F32, tag=f"rl{half}", name=f"rl{half}")
                nc.scalar.activation(rl[:, :], pd[:, :], AF.Relu)
                MUL = mybir.AluOpType.mult
                ADD = mybir.AluOpType.add
                SUB = mybir.AluOpType.subtract
                gacc = spool.tile([1, 128], F32, tag=f"ga{half}", name=f"ga{half}")
                gtmp = spool.tile([1, 128], F32, tag=f"gt{half}", name=f"gt{half}")
                # Horner: ((((s/5+1/4)s+1/3)s+1/2)s+1)s
                nc.vector.tensor_scalar(gacc[:, :], sS[:, :], 0.2, 0.25,
                                        MUL, ADD)
                nc.vector.tensor_tensor(gtmp[:, :], gacc[:, :], sS[:, :],
                                        op=MUL)
                nc.vector.tensor_scalar_add(gacc[:, :], gtmp[:, :], 1.0 / 3)
                nc.vector.tensor_tensor(gtmp[:, :], gacc[:, :], sS[:, :],
                                        op=MUL)
                nc.vector.tensor_scalar_add(gacc[:, :], gtmp[:, :], 0.5)
                nc.vector.tensor_tensor(gtmp[:, :], gacc[:, :], sS[:, :],
                                        op=MUL)
                nc.vector.tensor_scalar_add(gacc[:, :], gtmp[:, :], 1.0)
                nc.vector.tensor_tensor(gtmp[:, :], gacc[:, :], sS[:, :],
                                        op=MUL)
                yv = ystage.rearrange("p (b i) -> p b i", i=2)
                hb = half * 128
                # y0 = -(g + relu(d));  y1 = d + y0
                nc.vector.scalar_tensor_tensor(
                    yv[:, hb:hb + 128, 0:1],
                    gtmp.rearrange("p (f o) -> p f o", o=1), -1.0,
                    rl.rearrange("p (f o) -> p f o", o=1),
                    MUL, SUB)
                nc.vector.tensor_tensor(
                    yv[:, hb:hb + 128, 1:2],
                    pd.rearrange("p (f o) -> p f o", o=1),
                    yv[:, hb:hb + 128, 0:1], op=ADD)
                nc.sync.dma_start(
                    y.rearrange("(h b) i -> h (b i)", h=2)[half:half + 1, :],
                    ystage[:, hb * 2:hb * 2 + 256])

    nc.compile()
    _prog_cache['nc'] = nc
    return nc


# ---------------------------------------------------------------------------
# Entry point
# ---------------------------------------------------------------------------
def kernel(**inputs):
    from concourse.bass_utils import run_bass_kernel_spmd

    H = _build_host_tensors(inputs)
    x = np.asarray(inputs['x'], np.float32)
    nc = _build_program()

    in_maps = []
    for c in range(NCORES):
        m = dict(H)
        m['xT'] = _make_xT(x[c * BLOC:(c + 1) * BLOC])
        in_maps.append(m)

    trace = bool(os.environ.get('KERNEL_TRACE'))
    tmpdir = None
    if trace:
        tmpdir = os.environ.get('KERNEL_TRACE_DIR') or None
        if tmpdir:
            os.makedirs(tmpdir, exist_ok=True)
    res = run_bass_kernel_spmd(nc, in_maps, list(range(NCORES)),
                               trace=trace, tmpdir=tmpdir)
    kernel.last_results = res
    out = np.concatenate([res.results[c]['y'] for c in range(NCORES)], axis=0)
    return out.astype(np.float32)


if __name__ == '__main__':
    # smoke test with random inputs shaped per spec
    rng = np.random.default_rng(0)
    shapes = {
        'x': (B, 3, 8, 8, 8, 8),
        'l1c0': (1, 8, 3, 3), 'l1c1': (3, 4, 8, 2), 'l1c2': (2, 4, 8, 2),
        'l1c3': (2, 4, 8, 2), 'l1c4': (2, 4, 8, 1), 'b1': (8, 4, 4, 4, 4),
        'l2c0': (1, 4, 8, 2), 'l2c1': (2, 2, 4, 2), 'l2c2': (2, 2, 4, 2),
        'l2c3': (2, 2, 4, 2), 'l2c4': (2, 2, 4, 1), 'b2': (4, 2, 2, 2, 2),
        'l3c0': (1, 2, 4, 2), 'l3c1': (2, 2, 2, 2), 'l3c2': (2, 2, 2, 2),
        'l3c3': (2, 2, 2, 2), 'l3c4': (2, 2, 2, 1), 'b3': (2, 2, 2, 2, 2),
        'W': (2, 32), 'bl': (2,),
    }
    ins = {k: rng.standard_normal(v).astype(np.float32) * 0.3
           for k, v in shapes.items()}
    print(kernel(**ins)[:4])
